# revision 14
# baseline (speedup 1.0000x reference)
"""DeepSpeedMLP (residual-add -> LayerNorm -> fc1 -> ReLU -> fc2 -> residual-add)
on 8 Trainium2 NeuronCores.

Strategy (tensor-parallel, as DeepSpeed does):
  - inter_w sharded column-wise [H, I/8], output_w row-wise [I/8, H] per core.
  - Every core computes LN for all T=4096 tokens, fc1/fc2 on its I-shard,
    producing a partial fc2 output; a bf16 ReduceScatter sums partials and
    leaves each core with 1/8 of the tokens, to which bias + residual are
    added in fp32.
  - Activations are staged host-side in transposed [H, T] layout so the
    contraction dim of fc1 lands on SBUF partitions without any on-chip
    transpose. LN statistics are computed with ones-vector matmuls on the
    TensorEngine; mean subtraction rides the matmul as an augmented K=1 row
    (weights = gamma@W1 columns, moving row = -mu); rstd is applied at PSUM
    eviction; gamma is folded into W1 host-side (constant folding);
    output_b rides fc2 as an augmented ones-row with outb/8.
  - Matmuls in bf16 (fp32 PSUM accumulation). Token dim processed in 8
    blocks of 512 so LN(b+1) overlaps fc1/fc2(b) and the 8 ReduceScatters
    overlap compute.
"""

import numpy as np
import ml_dtypes

import concourse.bass as bass
import concourse.mybir as mybir
import concourse.tile as tile
from concourse import bacc
from concourse.bass_utils import run_bass_kernel_spmd

BF16 = mybir.dt.bfloat16
F32 = mybir.dt.float32
NPBF16 = ml_dtypes.bfloat16

H = 4096
T = 4096
I_FULL = 16384
NCORES = 8
I_S = I_FULL // NCORES   # 2048 intermediate cols per core
NB = 8                   # token blocks
TB = T // NB             # 512 tokens per block
KC = H // 128            # 32 contraction chunks for fc1
IT = I_S // 128          # 16 i-tiles
HT = H // 512            # 8 output h-tiles
TT = TB // 128           # 4 token tiles per block in fc2
OWN = TB // NCORES       # 64 token rows owned per core per block
LN_EPS = 1e-5

_CACHE = {}


def _src_hash():
    import hashlib
    with open(__file__, "rb") as f:
        return int(hashlib.sha256(f.read()).hexdigest()[:8], 16)


def _vtag_shape(repeat, sim):
    return ((_src_hash() % 97) + 1, 2 * repeat + (1 if sim else 0) + 1)


def _build(repeat=1, sim=False):
    nc = bacc.Bacc("TRN2", target_bir_lowering=False, debug=False,
                   num_devices=NCORES)
    with tile.TileContext(nc) as tc:
        with tc.tile_pool(name="dram", bufs=1, space="DRAM") as dram:
            def ext_in(name, shape, dtype):
                return dram.tile(shape, dtype, kind="ExternalInput", name=name,
                                 uniquify=False)

            xt = ext_in("xt", [H, T], BF16)            # x^T
            rt = ext_in("rt", [H, T], BF16)            # residual^T
            w1t = ext_in("w1t", [IT, 128, KC, 128], BF16)   # gamma-folded W1 shard
            w2t = ext_in("w2t", [HT, 128, IT, 512], BF16)   # W2 shard
            gw1 = ext_in("gw1", [1, I_S], BF16)        # gamma @ W1 shard
            biasf = ext_in("biasf", [128, IT], F32)    # beta@W1 + b1, per i-tile cols
            outb8 = ext_in("outb8", [1, H], BF16)      # output_b / 8
            xo = ext_in("xo", [NB * OWN, H], F32)      # owned x rows (block-major)
            ro = ext_in("ro", [NB * OWN, H], F32)      # owned residual rows
            # cache-busting tag: shape encodes source hash + build params
            # (the neuron compile cache keys on HLO shapes, not the BIR)
            vts = _vtag_shape(repeat, sim)
            vtag = ext_in("vtag", list(vts), F32)
            vscr = dram.tile(list(vts), F32, name="vscr")
            out = dram.tile([NB * OWN, H], F32, kind="ExternalOutput",
                            name="out", uniquify=False)

            # uneven RS split: big early chunk (hidden under compute),
            # smaller late chunks (short tail)
            RS_BLKS = [4, 2, 2]
            RS_TOK = [n * TB for n in RS_BLKS]
            RS_OWN = [t // NCORES for t in RS_TOK]
            NRS = len(RS_BLKS)
            RS_START = [sum(RS_BLKS[:h]) for h in range(NRS)]
            rsin = [dram.tile([RS_TOK[h], H], BF16, name=f"rsin{h}")
                    for h in range(NRS)]
            rsout = [dram.tile([RS_OWN[h], H], BF16, name=f"rsout{h}")
                     for h in range(NRS)]

            from contextlib import ExitStack
            ctx = ExitStack()
            with ctx:
                pool = lambda name, bufs, **kw: ctx.enter_context(
                    tc.tile_pool(name=name, bufs=bufs, **kw))
                consts = pool("consts", 1)
                hpool = pool("hpool", 2)
                ipool = pool("ipool", 2)
                lnst = pool("lnst", 3)
                h2p = pool("h2p", 3)
                w1p = pool("w1p", 2)
                w2p = pool("w2p", 2)
                evp = pool("evp", 3)
                ev2p = pool("ev2p", 3)
                bcp = pool("bcp", 2)
                rows = pool("rows", 1)
                finp = pool("finp", 1)
                psst = pool("psst", 1, space="PSUM")
                rsps = pool("rsps", 2, space="PSUM")
                fc1ps = pool("fc1ps", 2, space="PSUM")
                fc2ps = pool("fc2ps", 2, space="PSUM")
                ones_col = consts.tile([128, 1], BF16)
                nc.vector.memset(ones_col[:], 1.0)
                ones_row = consts.tile([1, 128], BF16)
                nc.vector.memset(ones_row[:], 1.0)
                ones_row_f = consts.tile([1, 128], F32)
                nc.vector.memset(ones_row_f[:], 1.0)
                eps_t = consts.tile([1, 1], F32)
                nc.vector.memset(eps_t[:], LN_EPS)
                gw1_sb = consts.tile([1, I_S], BF16)
                nc.sync.dma_start(out=gw1_sb[:], in_=gw1[:])
                biasf_sb = consts.tile([128, IT], F32)
                nc.sync.dma_start(out=biasf_sb[:], in_=biasf[:])
                outb8_sb = consts.tile([1, H], BF16)
                nc.sync.dma_start(out=outb8_sb[:], in_=outb8[:])
                nc.sync.dma_start(out=vscr[:], in_=vtag[:])

                for rep in range(repeat):
                  for b in range(NB):
                    ts = slice(b * TB, (b + 1) * TB)

                    # ---------- residual add + LN stats ----------
                    hT = hpool.tile([128, KC, TB], BF16, name="hT")
                    ps_s1 = psst.tile([1, TB], F32, name="ps_s1")
                    ps_s2 = psst.tile([1, TB], F32, name="ps_s2")
                    for kc in range(KC):
                        ks = slice(kc * 128, (kc + 1) * 128)
                        xt_t = lnst.tile([128, TB], BF16, name="xt_t")
                        rt_t = lnst.tile([128, TB], BF16, name="rt_t")
                        nc.sync.dma_start(out=xt_t[:], in_=xt[ks, ts])
                        nc.sync.dma_start(out=rt_t[:], in_=rt[ks, ts])
                        nc.vector.tensor_add(hT[:, kc, :], xt_t[:], rt_t[:])
                        h2_t = h2p.tile([128, TB], BF16, name="h2_t")
                        nc.scalar.square(out=h2_t[:], in_=hT[:, kc, :])
                        nc.tensor.matmul(ps_s1[:], ones_col[:], hT[:, kc, :],
                                         start=(kc == 0), stop=(kc == KC - 1))
                        nc.tensor.matmul(ps_s2[:], ones_col[:], h2_t[:],
                                         start=(kc == 0), stop=(kc == KC - 1))

                    murow = rows.tile([1, TB], F32, name="murow")
                    nc.scalar.mul(out=murow[:], in_=ps_s1[:], mul=1.0 / H)
                    s2row = rows.tile([1, TB], F32, name="s2row")
                    nc.scalar.mul(out=s2row[:], in_=ps_s2[:], mul=1.0 / H)
                    negmu = rows.tile([1, TB], BF16, name="negmu")
                    nc.scalar.mul(out=negmu[:], in_=murow[:], mul=-1.0)
                    varrow = rows.tile([1, TB], F32, name="varrow")
                    nc.vector.tensor_mul(varrow[:], murow[:], murow[:])
                    nc.vector.tensor_sub(varrow[:], s2row[:], varrow[:])
                    # std = sqrt(var + eps), rstd = 1/std
                    nc.scalar.activation(out=varrow[:], in_=varrow[:],
                                         func=mybir.ActivationFunctionType.Sqrt,
                                         bias=eps_t[:])
                    rstd_row = rows.tile([1, TB], F32, name="rstd_row")
                    nc.vector.reciprocal(out=rstd_row[:], in_=varrow[:])
                    # broadcast rstd across partitions via K=1 outer product
                    rstd_ps = rsps.tile([128, TB], F32, name="rstd_ps")
                    nc.tensor.matmul(rstd_ps[:], ones_row_f[:], rstd_row[:],
                                     start=True, stop=True)
                    rstd_bc = bcp.tile([128, TB], F32, name="rstd_bc")
                    nc.scalar.copy(out=rstd_bc[:], in_=rstd_ps[:])

                    # ---------- fc1: interT[i, t] ----------
                    interT = ipool.tile([128, IT, TB], BF16, name="interT")
                    for it in range(IT):
                        w1_t = w1p.tile([128, KC, 128], BF16, name="w1_t")
                        nc.sync.dma_start(out=w1_t[:], in_=w1t[it])
                        ps1 = fc1ps.tile([128, TB], F32, name="ps1")
                        for kc in range(KC):
                            nc.tensor.matmul(ps1[:], w1_t[:, kc, :], hT[:, kc, :],
                                             start=(kc == 0), stop=False)
                        nc.tensor.matmul(ps1[:],
                                         gw1_sb[:, it * 128:(it + 1) * 128],
                                         negmu[:], start=False, stop=True)
                        tmp = evp.tile([128, TB], BF16, name="tmp")
                        nc.vector.tensor_mul(tmp[:], ps1[:], rstd_bc[:])
                        nc.scalar.activation(out=interT[:, it, :], in_=tmp[:],
                                             func=mybir.ActivationFunctionType.Relu,
                                             bias=biasf_sb[:, it:it + 1])

                    # ---------- fc2: partial[t, h] ----------
                    for ht in range(HT):
                        w2_th = [None, None]
                        for half in range(2):
                            w2_t = w2p.tile([128, IT // 2, 512], BF16,
                                            name="w2_t")
                            nc.sync.dma_start(
                                out=w2_t[:],
                                in_=w2t[ht, :, half * (IT // 2):(half + 1) * (IT // 2), :])
                            w2_th[half] = w2_t
                        for tt in range(TT):
                            ps2 = fc2ps.tile([128, 512], F32, name="ps2")
                            for ic in range(IT):
                                nc.tensor.matmul(
                                    ps2[:],
                                    interT[:, ic, tt * 128:(tt + 1) * 128],
                                    w2_th[ic // (IT // 2)][:, ic % (IT // 2), :],
                                    start=(ic == 0), stop=False)
                            nc.tensor.matmul(ps2[:], ones_row[:],
                                             outb8_sb[:, ht * 512:(ht + 1) * 512],
                                             start=False, stop=True)
                            ev2 = ev2p.tile([128, 512], BF16, name="ev2")
                            nc.scalar.copy(out=ev2[:], in_=ps2[:])
                            half = max(i for i in range(NRS)
                                       if RS_START[i] <= b)
                            ro0 = (b - RS_START[half]) * TB + tt * 128
                            nc.scalar.dma_start(
                                out=rsin[half][ro0:ro0 + 128,
                                               ht * 512:(ht + 1) * 512],
                                in_=ev2[:])

                    # ---------- cross-core reduce: one RS per half ----------
                    def emit_rs(h):
                        if sim:
                            for ck in range(0, RS_OWN[h], 128):
                                nc.sync.dma_start(
                                    out=rsout[h][ck:ck + 128, :],
                                    in_=rsin[h][ck:ck + 128, :])
                        else:
                            nc.gpsimd.collective_compute(
                                "ReduceScatter",
                                mybir.AluOpType.add,
                                replica_groups=[list(range(NCORES))],
                                ins=[rsin[h].opt()],
                                outs=[rsout[h].opt()],
                            )

                    def emit_final(h):
                        for ck in range(RS_OWN[h] // 128):
                            rs0 = ck * 128
                            oo0 = sum(RS_OWN[:h]) + ck * 128
                            for hs in range(4):
                                cs = slice(hs * 1024, (hs + 1) * 1024)
                                rs_t = finp.tile([128, 1024], BF16, name="rs_t")
                                nc.gpsimd.dma_start(
                                    out=rs_t[:],
                                    in_=rsout[h][rs0:rs0 + 128, cs])
                                xo_t = finp.tile([128, 1024], F32, name="xo_t")
                                nc.sync.dma_start(out=xo_t[:],
                                                  in_=xo[oo0:oo0 + 128, cs])
                                ro_t = finp.tile([128, 1024], F32, name="ro_t")
                                nc.sync.dma_start(out=ro_t[:],
                                                  in_=ro[oo0:oo0 + 128, cs])
                                s1_t = finp.tile([128, 1024], F32, name="s1_t")
                                nc.vector.tensor_add(s1_t[:], xo_t[:], ro_t[:])
                                s2_t = finp.tile([128, 1024], F32, name="s2_t")
                                nc.vector.tensor_add(s2_t[:], s1_t[:], rs_t[:])
                                nc.sync.dma_start(out=out[oo0:oo0 + 128, cs],
                                                  in_=s2_t[:])

                    for h in range(NRS):
                        if b == RS_START[h] + RS_BLKS[h] - 1:
                            emit_rs(h)
                    if b == NB - 2:
                        emit_final(0)    # RS(0) long done by now
                    elif b == NB - 1:
                        for h in range(1, NRS):
                            emit_final(h)
    nc.compile()
    return nc


def _own_idx(c):
    # RS chunks of [4,2,2] blocks; rank c owns rows [c*own, (c+1)*own) of each
    parts = []
    start = 0
    for nblk in (4, 2, 2):
        tok = nblk * TB
        own = tok // NCORES
        parts.append(start + c * own + np.arange(own))
        start += tok
    return np.concatenate(parts)


def _prep_inputs(x, residual, gamma, beta, inter_w, inter_b, output_w, output_b):
    f32 = np.float32
    x3 = np.ascontiguousarray(np.asarray(x, dtype=f32).reshape(T, H))
    r3 = np.ascontiguousarray(np.asarray(residual, dtype=f32).reshape(T, H))
    gamma = np.asarray(gamma, dtype=f32)
    beta = np.asarray(beta, dtype=f32)
    inter_w = np.asarray(inter_w, dtype=f32)
    inter_b = np.asarray(inter_b, dtype=f32)
    output_w = np.asarray(output_w, dtype=f32)
    output_b = np.asarray(output_b, dtype=f32)

    xt_np = np.ascontiguousarray(x3.T).astype(NPBF16)
    rt_np = np.ascontiguousarray(r3.T).astype(NPBF16)

    w1f = inter_w * gamma[:, None]
    gw1_full = gamma @ inter_w
    bias_full = beta @ inter_w + inter_b
    outb8_np = (output_b / NCORES).astype(NPBF16).reshape(1, H)

    in_maps = []
    for c in range(NCORES):
        sl = slice(c * I_S, (c + 1) * I_S)
        w1s = w1f[:, sl]
        # [IT, 128(k in chunk), KC, 128(i)]
        w1tiles = np.ascontiguousarray(
            w1s.reshape(KC, 128, IT, 128).transpose(2, 1, 0, 3)).astype(NPBF16)
        w2s = output_w[sl, :]
        # [HT, 128(i in chunk), IT, 512(h)]
        w2tiles = np.ascontiguousarray(
            w2s.reshape(IT, 128, HT, 512).transpose(2, 1, 0, 3)).astype(NPBF16)
        gw1_c = gw1_full[sl].astype(NPBF16).reshape(1, I_S)
        biasf_c = np.ascontiguousarray(
            bias_full[sl].reshape(IT, 128).T).astype(f32)
        idx = _own_idx(c)
        in_maps.append({
            "vtag": np.zeros(_vtag_shape(1, False), dtype=f32),
            "xt": xt_np, "rt": rt_np,
            "w1t": w1tiles, "w2t": w2tiles,
            "gw1": gw1_c, "biasf": biasf_c, "outb8": outb8_np,
            "xo": np.ascontiguousarray(x3[idx]),
            "ro": np.ascontiguousarray(r3[idx]),
        })
    return in_maps


def get_nc(repeat=1, sim=False):
    key = ("nc", repeat, sim)
    if key not in _CACHE:
        _CACHE[key] = _build(repeat=repeat, sim=sim)
    return _CACHE[key]


def run(in_maps):
    nc = get_nc()
    return run_bass_kernel_spmd(nc, in_maps, core_ids=list(range(NCORES)))


def kernel(x, residual, gamma, beta, inter_w, inter_b, output_w, output_b):
    in_maps = _prep_inputs(x, residual, gamma, beta, inter_w, inter_b,
                           output_w, output_b)
    res = run(in_maps)
    out_full = np.empty((T, H), dtype=np.float32)
    for c in range(NCORES):
        out_full[_own_idx(c)] = res.results[c]["out"]
    return out_full.reshape(2, T // 2, H)
